# revision 2
# baseline (speedup 1.0000x reference)
"""Trainium2 Bass kernel for nn_CubECLayr: Euler characteristic curves of
sublevel cubical complexes, batch-data-parallel over 8 NeuronCores.

Algorithm (per core, 24 images of 256x256, 3 chunks of 8 images):
  1. k = ceil(x / DT) per pixel (exact integer bin, magic-number round).
  2. Vertex attribution: signed cell-count delta d per pixel from neighbor
     comparisons in k-space (|d| <= 3); ECC_s = sum_p d_p * [k_p <= s].
  3. Threshold engine via the ramp identity: with y = k + d/8 (exact f16),
         F_v(c) = sum_j min(v_j - c, 0),   c_s = s + 1/2
         ECC_s  = 8 * (F_y(c_s) - F_k(c_s))
     Both families are single-source 4x-mode tensor_scalar passes.
     Reduction is done off-DVE:
       - route A (24 thresholds): PE matmul with +-block-diagonal weights
         sums (w_y - w_k) per image into PSUM [8,512]; ACT drains to [8,1].
       - route C (8 thresholds): ACT Relu+accum computes per-partition
         -F directly (sum relu(c - v)); a final PE matmul reduces to images.
  All intermediate values are integer multiples of 1/8 bounded well below
  2^24/8, so every sum is exact in f16/f32.
"""

import numpy as np

import concourse.bacc as bacc
import concourse.mybir as mybir
from concourse import tile
from concourse.bass_utils import run_bass_kernel_spmd

NCORES = 8
B, C, H, W = 64, 3, 256, 256
IMGS = (B // NCORES) * C          # 24 images per core
CHUNK_IMGS = 8
NCHUNK = IMGS // CHUNK_IMGS       # 3
RB = 16                           # partitions per image
ROWS = H // RB                    # 16 own rows per partition
FD = ROWS * W                     # 4096 own pixels per partition
STEPS = 32
PAD = 1000.0                      # > any real bin; exact in f16
MAGIC = 8388608.0                 # 2^23
HALF = float(np.float32(0.49999997))
F32 = mybir.dt.float32
F16 = mybir.dt.float16
Op = mybir.AluOpType
Act = mybir.ActivationFunctionType

# threshold routing: C-route thresholds handled by ACT alone
C_SET = [3, 7, 11, 15, 19, 23, 27, 31]
A_SET = [s for s in range(STEPS) if s not in C_SET]
NC_ROUTE = len(C_SET)

_NC_CACHE = {}


def _build_nc():
    nc = bacc.Bacc(None, target_bir_lowering=False)
    x_in = nc.dram_tensor("x", [NCHUNK * 128, FD], F32, kind="ExternalInput")
    # bdp: +1 block-diag (f16); bdn: -1 block-diag (f16); bdf: +1 (f32)
    bdp_in = nc.dram_tensor("bdp", [128, CHUNK_IMGS], F32, kind="ExternalInput")
    bdn_in = nc.dram_tensor("bdn", [128, CHUNK_IMGS], F32, kind="ExternalInput")
    # per-threshold ACT bias columns: cb[:, s] = s + 0.5
    cb_in = nc.dram_tensor("cb", [128, STEPS], F32, kind="ExternalInput")
    out = nc.dram_tensor("out", [NCHUNK * CHUNK_IMGS, STEPS], F32,
                         kind="ExternalOutput")

    with tile.TileContext(nc) as tc:
        with (
            tc.tile_pool(name="xp", bufs=2) as xp,
            tc.tile_pool(name="wp", bufs=1) as wp,
            tc.tile_pool(name="rp", bufs=4) as rp,
            tc.tile_pool(name="cst", bufs=1) as cst,
            tc.tile_pool(name="pp", bufs=4, space="PSUM") as pp,
            tc.tile_pool(name="pc", bufs=2, space="PSUM") as pc,
        ):
            bdp = cst.tile([128, CHUNK_IMGS], F16)
            bdn = cst.tile([128, CHUNK_IMGS], F16)
            bdf = cst.tile([128, CHUNK_IMGS], F32)
            cb = cst.tile([128, STEPS], F32)
            tmpf = cst.tile([128, CHUNK_IMGS], F32)
            nc.sync.dma_start(out=tmpf[:], in_=bdp_in[:])
            nc.vector.tensor_copy(out=bdp[:], in_=tmpf[:])
            nc.vector.tensor_copy(out=bdf[:], in_=tmpf[:])
            tmpf2 = cst.tile([128, CHUNK_IMGS], F32)
            nc.sync.dma_start(out=tmpf2[:], in_=bdn_in[:])
            nc.vector.tensor_copy(out=bdn[:], in_=tmpf2[:])
            nc.sync.dma_start(out=cb[:], in_=cb_in[:])
            padt = cst.tile([CHUNK_IMGS, W], F16)
            nc.vector.memset(padt[:], PAD)
            adump = cst.tile([128, FD], F16)   # ACT full-output dump
            ddump = cst.tile([8, 512], F32)    # drain dump

            for c in range(NCHUNK):
                xt = xp.tile([128, FD], F32, tag="xt")
                nc.sync.dma_start(out=xt[:], in_=x_in[c * 128:(c + 1) * 128, :])

                # --- bins: k = round(x*31 + (0.5 - eps)) == ceil(x/DT) ---
                nc.vector.tensor_scalar(
                    out=xt[:], in0=xt[:], scalar1=31.0, scalar2=HALF,
                    op0=Op.mult, op1=Op.add)
                # kt rows: 0 = top halo, 1..16 own, 17 = bottom halo (flat cols)
                kt = wp.tile([128, (ROWS + 2) * W], F16, tag="kt")
                nc.vector.tensor_scalar(
                    out=kt[:, W:W + FD], in0=xt[:], scalar1=MAGIC, scalar2=-MAGIC,
                    op0=Op.add, op1=Op.add)
                # halo exchange between partitions (same image), pad at borders
                nc.vector.memset(kt[:, 0:W], PAD)
                nc.vector.memset(kt[:, FD + W:FD + 2 * W], PAD)
                nc.gpsimd.dma_start(out=kt[1:128, 0:W], in_=kt[0:127, FD:FD + W])
                nc.gpsimd.dma_start(out=kt[0:127, FD + W:FD + 2 * W],
                                    in_=kt[1:128, W:2 * W])
                ktop = kt[:, 0:W].rearrange("(a b) w -> a b w", b=RB)
                nc.gpsimd.dma_start(out=ktop[:, 0, :], in_=padt[:])
                kbot = kt[:, FD + W:FD + 2 * W].rearrange("(a b) w -> a b w", b=RB)
                nc.gpsimd.dma_start(out=kbot[:, RB - 1, :], in_=padt[:])

                # --- neighbor comparisons (k-space) ---
                rh = wp.tile([128, FD], F16, tag="rh")
                nc.vector.tensor_tensor(
                    out=rh[:], in0=kt[:, W + 1:W + FD + 1], in1=kt[:, W:W + FD],
                    op=Op.is_ge)
                rv = wp.tile([128, (ROWS + 1) * W], F16, tag="rv")
                nc.vector.tensor_tensor(
                    out=rv[:], in0=kt[:, W:], in1=kt[:, 0:(ROWS + 1) * W],
                    op=Op.is_ge)
                khe = wp.tile([128, (ROWS + 2) * W], F16, tag="khe")
                nc.vector.tensor_tensor(
                    out=khe[:, 0:(ROWS + 2) * W - 1],
                    in0=kt[:, 0:(ROWS + 2) * W - 1], in1=kt[:, 1:(ROWS + 2) * W],
                    op=Op.max)
                nc.vector.memset(
                    khe[:, (ROWS + 2) * W - 1:(ROWS + 2) * W], PAD)
                ut = wp.tile([128, (ROWS + 1) * W], F16, tag="ut")
                nc.vector.tensor_tensor(
                    out=ut[:], in0=khe[:, W:], in1=khe[:, 0:(ROWS + 1) * W],
                    op=Op.is_ge)
                cc = wp.tile([128, FD], F16, tag="cc")
                nc.vector.tensor_tensor(
                    out=cc[:], in0=ut[:, W:], in1=ut[:, 0:FD], op=Op.subtract)
                cc3 = cc[:].rearrange("p (r w) -> p r w", w=W)
                nc.vector.memset(cc3[:, :, W - 1:W], 0.0)
                tt = wp.tile([128, FD], F16, tag="tt")
                nc.vector.tensor_tensor(out=tt[:], in0=rh[:], in1=cc[:], op=Op.mult)

                # --- delta assembly: dl = rv_b - rv_a + t - t_left - cc ---
                dl = wp.tile([128, FD], F16, tag="dl")
                nc.vector.tensor_tensor(
                    out=dl[:], in0=rv[:, W:], in1=rv[:, 0:FD], op=Op.subtract)
                nc.vector.tensor_tensor(out=dl[:], in0=dl[:], in1=tt[:], op=Op.add)
                nc.vector.tensor_tensor(
                    out=dl[:, 1:FD], in0=dl[:, 1:FD], in1=tt[:, 0:FD - 1],
                    op=Op.subtract)
                nc.vector.tensor_tensor(
                    out=dl[:], in0=dl[:], in1=cc[:], op=Op.subtract)

                # --- y = k + d/8 (exact in f16) ---
                yt = wp.tile([128, FD], F16, tag="yt")
                nc.vector.tensor_scalar(
                    out=yt[:], in0=dl[:], scalar1=0.125, scalar2=0.0,
                    op0=Op.mult, op1=Op.add)
                nc.vector.tensor_tensor(
                    out=yt[:], in0=yt[:], in1=kt[:, W:W + FD], op=Op.add)
                kv = kt[:, W:W + FD]

                # --- output tile for this chunk ---
                ft = wp.tile([8, STEPS], F32, tag="ft")
                # C-route accum columns: [y0..y7 | k0..k7]
                colt = wp.tile([128, 2 * NC_ROUTE], F32, tag="colt")

                # --- route A: DVE ramp -> PE +-bd matmul -> ACT drain ---
                for s in A_SET:
                    cs = s + 0.5
                    wy = rp.tile([128, FD], F16, tag="wy")
                    nc.vector.tensor_scalar(
                        out=wy[:], in0=yt[:], scalar1=-cs, scalar2=0.0,
                        op0=Op.add, op1=Op.min)
                    wk = rp.tile([128, FD], F16, tag="wk")
                    nc.vector.tensor_scalar(
                        out=wk[:], in0=kv, scalar1=-cs, scalar2=0.0,
                        op0=Op.add, op1=Op.min)
                    psum = pp.tile([8, 512], F32, tag="ps")
                    wy3 = wy[:].rearrange("p (h n) -> p h n", n=512)
                    wk3 = wk[:].rearrange("p (h n) -> p h n", n=512)
                    for h in range(8):
                        nc.tensor.matmul(psum[:], bdp[:], wy3[:, h, :],
                                         start=(h == 0), stop=False)
                    for h in range(8):
                        nc.tensor.matmul(psum[:], bdn[:], wk3[:, h, :],
                                         start=False, stop=(h == 7))
                    nc.scalar.activation(
                        out=ddump[:], in_=psum[:], func=Act.Copy,
                        accum_out=ft[:, s:s + 1])

                # --- route C: ACT Relu+accum (sum relu(c - v) = -F_v) ---
                for i, s in enumerate(C_SET):
                    nc.scalar.activation(
                        out=adump[:], in_=yt[:], func=Act.Relu,
                        bias=cb[:, s:s + 1], scale=-1.0,
                        accum_out=colt[:, i:i + 1])
                    nc.scalar.activation(
                        out=adump[:], in_=kv, func=Act.Relu,
                        bias=cb[:, s:s + 1], scale=-1.0,
                        accum_out=colt[:, NC_ROUTE + i:NC_ROUTE + i + 1])
                # D = colK - colY = F_y - F_k  (per partition)
                dcol = wp.tile([128, NC_ROUTE], F32, tag="dcol")
                nc.vector.tensor_tensor(
                    out=dcol[:], in0=colt[:, NC_ROUTE:2 * NC_ROUTE],
                    in1=colt[:, 0:NC_ROUTE], op=Op.subtract)
                psc = pc.tile([8, NC_ROUTE], F32, tag="psc")
                nc.tensor.matmul(psc[:], bdf[:], dcol[:], start=True, stop=True)
                ft4 = ft[:].rearrange("p (a b) -> p a b", b=4)
                nc.scalar.activation(
                    out=ft4[:, :, 3], in_=psc[:], func=Act.Copy)

                nc.sync.dma_start(
                    out=out[c * CHUNK_IMGS:(c + 1) * CHUNK_IMGS, :], in_=ft[:])

    nc.finalize()
    return nc


def _host_consts():
    bdp = np.zeros((128, CHUNK_IMGS), dtype=np.float32)
    for p in range(128):
        bdp[p, p // RB] = 1.0
    bdn = -bdp
    cb = np.tile((np.arange(STEPS, dtype=np.float32) + 0.5)[None, :], (128, 1))
    return bdp, bdn, cb


def kernel(x: np.ndarray) -> np.ndarray:
    assert x.shape == (B, C, H, W) and x.dtype == np.float32
    if "nc" not in _NC_CACHE:
        _NC_CACHE["nc"] = _build_nc()
    nc = _NC_CACHE["nc"]

    bdp, bdn, cb = _host_consts()
    in_maps = []
    for i in range(NCORES):
        shard = x[i * (B // NCORES):(i + 1) * (B // NCORES)]  # (8, 3, 256, 256)
        in_maps.append({
            "x": np.ascontiguousarray(shard).reshape(NCHUNK * 128, FD),
            "bdp": bdp, "bdn": bdn, "cb": cb,
        })
    res = run_bass_kernel_spmd(nc, in_maps, core_ids=list(range(NCORES)))
    parts = []
    for i in range(NCORES):
        f = res.results[i]["out"].astype(np.float64)  # (24, 32) = F_y - F_k
        acc = 8.0 * f
        parts.append(acc.reshape(B // NCORES, C, STEPS))
    return np.concatenate(parts, axis=0).reshape(B, C * STEPS).astype(np.float32)


if __name__ == "__main__":
    rng = np.random.default_rng(0)
    x = rng.random((B, C, H, W), dtype=np.float32)
    y = kernel(x)
    print("kernel out", y.shape, y.dtype, y[:2, :6])


# revision 3
# speedup vs baseline: 1.0811x; 1.0811x over previous
"""Trainium2 Bass kernel for nn_CubECLayr: Euler characteristic curves of
sublevel cubical complexes, batch-data-parallel over 8 NeuronCores.

Algorithm (per core, 24 images of 256x256, 3 chunks of 8 images):
  1. k = ceil(x / DT) per pixel (exact integer bin, magic-number round).
  2. Vertex attribution: signed cell-count delta d per pixel from neighbor
     comparisons in k-space (|d| <= 3); ECC_s = sum_p d_p * [k_p <= s].
  3. Threshold engine via the ramp identity: with y = k + d/8 (exact f16),
         F_v(c) = sum_j min(v_j - c, 0),   c_s = s + 1/2
         ECC_s  = 8 * (F_y(c_s) - F_k(c_s))
     Both families are single-source 4x-mode tensor_scalar passes.
     Reduction is done off-DVE:
       - route A (24 thresholds): PE matmul with +-block-diagonal weights
         sums (w_y - w_k) per image into PSUM [8,512]; ACT drains to [8,1].
       - route C (8 thresholds): ACT Relu+accum computes per-partition
         -F directly (sum relu(c - v)); a final PE matmul reduces to images.
  All intermediate values are integer multiples of 1/8 bounded well below
  2^24/8, so every sum is exact in f16/f32.
"""

import numpy as np

import concourse.bacc as bacc
import concourse.mybir as mybir
from concourse import tile
from concourse.bass_utils import run_bass_kernel_spmd

NCORES = 8
B, C, H, W = 64, 3, 256, 256
IMGS = (B // NCORES) * C          # 24 images per core
CHUNK_IMGS = 8
NCHUNK = IMGS // CHUNK_IMGS       # 3
RB = 16                           # partitions per image
ROWS = H // RB                    # 16 own rows per partition
FD = ROWS * W                     # 4096 own pixels per partition
STEPS = 32
PAD = 1000.0                      # > any real bin; exact in f16
MAGIC = 8388608.0                 # 2^23
HALF = float(np.float32(0.49999997))
F32 = mybir.dt.float32
F16 = mybir.dt.float16
Op = mybir.AluOpType
Act = mybir.ActivationFunctionType

# threshold routing: C-route thresholds handled by ACT alone
C_SET = [3, 7, 11, 15, 19, 23, 27, 31]
A_SET = [s for s in range(STEPS) if s not in C_SET]
NC_ROUTE = len(C_SET)

_NC_CACHE = {}


def _build_nc():
    nc = bacc.Bacc(None, target_bir_lowering=False)
    x_in = nc.dram_tensor("x", [NCHUNK * 128, FD], F32, kind="ExternalInput")
    # bdp: +1 block-diag (f16); bdn: -1 block-diag (f16); bdf: +1 (f32)
    bdp_in = nc.dram_tensor("bdp", [128, CHUNK_IMGS], F32, kind="ExternalInput")
    bdn_in = nc.dram_tensor("bdn", [128, CHUNK_IMGS], F32, kind="ExternalInput")
    # per-threshold ACT bias columns: cb[:, s] = s + 0.5
    cb_in = nc.dram_tensor("cb", [128, STEPS], F32, kind="ExternalInput")
    out = nc.dram_tensor("out", [NCHUNK * CHUNK_IMGS, STEPS], F32,
                         kind="ExternalOutput")

    with tile.TileContext(nc) as tc:
        with (
            tc.tile_pool(name="xp", bufs=2) as xp,
            tc.tile_pool(name="wp", bufs=1) as wp,
            tc.tile_pool(name="hp", bufs=2) as hp,
            tc.tile_pool(name="rp", bufs=4) as rp,
            tc.tile_pool(name="cst", bufs=1) as cst,
            tc.tile_pool(name="pp", bufs=4, space="PSUM") as pp,
            tc.tile_pool(name="pc", bufs=2, space="PSUM") as pc,
        ):
            bdp = cst.tile([128, CHUNK_IMGS], F16)
            bdn = cst.tile([128, CHUNK_IMGS], F16)
            bdf = cst.tile([128, CHUNK_IMGS], F32)
            cb = cst.tile([128, STEPS], F32)
            tmpf = cst.tile([128, CHUNK_IMGS], F32)
            nc.sync.dma_start(out=tmpf[:], in_=bdp_in[:])
            nc.vector.tensor_copy(out=bdp[:], in_=tmpf[:])
            nc.vector.tensor_copy(out=bdf[:], in_=tmpf[:])
            tmpf2 = cst.tile([128, CHUNK_IMGS], F32)
            nc.sync.dma_start(out=tmpf2[:], in_=bdn_in[:])
            nc.vector.tensor_copy(out=bdn[:], in_=tmpf2[:])
            nc.sync.dma_start(out=cb[:], in_=cb_in[:])
            padt = cst.tile([CHUNK_IMGS, W], F16)
            nc.vector.memset(padt[:], PAD)
            adump = cst.tile([128, FD], F16)   # ACT full-output dump
            ddump = cst.tile([8, 512], F32)    # drain dump

            for c in range(NCHUNK):
                xt = xp.tile([128, FD], F32, tag="xt")
                nc.sync.dma_start(out=xt[:], in_=x_in[c * 128:(c + 1) * 128, :])

                # --- bins: k = round(x*31 + (0.5 - eps)) == ceil(x/DT) ---
                nc.vector.tensor_scalar(
                    out=xt[:], in0=xt[:], scalar1=31.0, scalar2=HALF,
                    op0=Op.mult, op1=Op.add)
                # kt rows: 0 = top halo, 1..16 own, 17 = bottom halo (flat cols)
                kt = hp.tile([128, (ROWS + 2) * W], F16, tag="kt")
                nc.vector.tensor_scalar(
                    out=kt[:, W:W + FD], in0=xt[:], scalar1=MAGIC, scalar2=-MAGIC,
                    op0=Op.add, op1=Op.add)
                # halo exchange between partitions (same image), pad at borders
                nc.gpsimd.memset(kt[:, 0:W], PAD)
                nc.gpsimd.memset(kt[:, FD + W:FD + 2 * W], PAD)
                nc.gpsimd.dma_start(out=kt[1:128, 0:W], in_=kt[0:127, FD:FD + W])
                nc.gpsimd.dma_start(out=kt[0:127, FD + W:FD + 2 * W],
                                    in_=kt[1:128, W:2 * W])
                ktop = kt[:, 0:W].rearrange("(a b) w -> a b w", b=RB)
                nc.gpsimd.dma_start(out=ktop[:, 0, :], in_=padt[:])
                kbot = kt[:, FD + W:FD + 2 * W].rearrange("(a b) w -> a b w", b=RB)
                nc.gpsimd.dma_start(out=kbot[:, RB - 1, :], in_=padt[:])

                # --- neighbor comparisons (k-space) ---
                rh = wp.tile([128, FD], F16, tag="rh")
                nc.vector.tensor_tensor(
                    out=rh[:], in0=kt[:, W + 1:W + FD + 1], in1=kt[:, W:W + FD],
                    op=Op.is_ge)
                rv = wp.tile([128, (ROWS + 1) * W], F16, tag="rv")
                nc.vector.tensor_tensor(
                    out=rv[:], in0=kt[:, W:], in1=kt[:, 0:(ROWS + 1) * W],
                    op=Op.is_ge)
                khe = wp.tile([128, (ROWS + 2) * W], F16, tag="khe")
                nc.vector.tensor_tensor(
                    out=khe[:, 0:(ROWS + 2) * W - 1],
                    in0=kt[:, 0:(ROWS + 2) * W - 1], in1=kt[:, 1:(ROWS + 2) * W],
                    op=Op.max)
                nc.gpsimd.memset(
                    khe[:, (ROWS + 2) * W - 1:(ROWS + 2) * W], PAD)
                ut = wp.tile([128, (ROWS + 1) * W], F16, tag="ut")
                nc.vector.tensor_tensor(
                    out=ut[:], in0=khe[:, W:], in1=khe[:, 0:(ROWS + 1) * W],
                    op=Op.is_ge)
                cc = wp.tile([128, FD], F16, tag="cc")
                nc.vector.tensor_tensor(
                    out=cc[:], in0=ut[:, W:], in1=ut[:, 0:FD], op=Op.subtract)
                cc3 = cc[:].rearrange("p (r w) -> p r w", w=W)
                nc.gpsimd.memset(cc3[:, :, W - 1:W], 0.0)
                tt = wp.tile([128, FD], F16, tag="tt")
                nc.vector.tensor_tensor(out=tt[:], in0=rh[:], in1=cc[:], op=Op.mult)

                # --- delta assembly: dl = rv_b - rv_a + t - t_left - cc ---
                dl = wp.tile([128, FD], F16, tag="dl")
                nc.vector.tensor_tensor(
                    out=dl[:], in0=rv[:, W:], in1=rv[:, 0:FD], op=Op.subtract)
                nc.vector.tensor_tensor(out=dl[:], in0=dl[:], in1=tt[:], op=Op.add)
                nc.vector.tensor_tensor(
                    out=dl[:, 1:FD], in0=dl[:, 1:FD], in1=tt[:, 0:FD - 1],
                    op=Op.subtract)
                nc.vector.tensor_tensor(
                    out=dl[:], in0=dl[:], in1=cc[:], op=Op.subtract)

                # --- y = k + d/8 (exact in f16) ---
                yt = hp.tile([128, FD], F16, tag="yt")
                nc.vector.tensor_scalar(
                    out=yt[:], in0=dl[:], scalar1=0.125, scalar2=0.0,
                    op0=Op.mult, op1=Op.add)
                nc.vector.tensor_tensor(
                    out=yt[:], in0=yt[:], in1=kt[:, W:W + FD], op=Op.add)
                kv = kt[:, W:W + FD]

                # --- output tile for this chunk ---
                ft = hp.tile([8, STEPS], F32, tag="ft")
                # C-route accum columns: [y0..y7 | k0..k7]
                colt = hp.tile([128, 2 * NC_ROUTE], F32, tag="colt")

                # --- route A: DVE ramp -> PE +-bd matmul -> ACT drain ---
                for s in A_SET:
                    cs = s + 0.5
                    wy = rp.tile([128, FD], F16, tag="wy")
                    nc.vector.tensor_scalar(
                        out=wy[:], in0=yt[:], scalar1=-cs, scalar2=0.0,
                        op0=Op.add, op1=Op.min)
                    wk = rp.tile([128, FD], F16, tag="wk")
                    nc.vector.tensor_scalar(
                        out=wk[:], in0=kv, scalar1=-cs, scalar2=0.0,
                        op0=Op.add, op1=Op.min)
                    psum = pp.tile([8, 512], F32, tag="ps")
                    wy3 = wy[:].rearrange("p (h n) -> p h n", n=512)
                    wk3 = wk[:].rearrange("p (h n) -> p h n", n=512)
                    for h in range(8):
                        nc.tensor.matmul(psum[:], bdp[:], wy3[:, h, :],
                                         start=(h == 0), stop=False)
                    for h in range(8):
                        nc.tensor.matmul(psum[:], bdn[:], wk3[:, h, :],
                                         start=False, stop=(h == 7))
                    nc.scalar.activation(
                        out=ddump[:], in_=psum[:], func=Act.Copy,
                        accum_out=ft[:, s:s + 1])

                # --- route C: ACT Relu+accum (sum relu(c - v) = -F_v) ---
                for i, s in enumerate(C_SET):
                    nc.scalar.activation(
                        out=adump[:], in_=yt[:], func=Act.Relu,
                        bias=cb[:, s:s + 1], scale=-1.0,
                        accum_out=colt[:, i:i + 1])
                    nc.scalar.activation(
                        out=adump[:], in_=kv, func=Act.Relu,
                        bias=cb[:, s:s + 1], scale=-1.0,
                        accum_out=colt[:, NC_ROUTE + i:NC_ROUTE + i + 1])
                # D = colK - colY = F_y - F_k  (per partition)
                dcol = hp.tile([128, NC_ROUTE], F32, tag="dcol")
                nc.vector.tensor_tensor(
                    out=dcol[:], in0=colt[:, NC_ROUTE:2 * NC_ROUTE],
                    in1=colt[:, 0:NC_ROUTE], op=Op.subtract)
                psc = pc.tile([8, NC_ROUTE], F32, tag="psc")
                nc.tensor.matmul(psc[:], bdf[:], dcol[:], start=True, stop=True)
                ft4 = ft[:].rearrange("p (a b) -> p a b", b=4)
                nc.scalar.activation(
                    out=ft4[:, :, 3], in_=psc[:], func=Act.Copy)

                nc.sync.dma_start(
                    out=out[c * CHUNK_IMGS:(c + 1) * CHUNK_IMGS, :], in_=ft[:])

    nc.finalize()
    return nc


def _host_consts():
    bdp = np.zeros((128, CHUNK_IMGS), dtype=np.float32)
    for p in range(128):
        bdp[p, p // RB] = 1.0
    bdn = -bdp
    cb = np.tile((np.arange(STEPS, dtype=np.float32) + 0.5)[None, :], (128, 1))
    return bdp, bdn, cb


def kernel(x: np.ndarray) -> np.ndarray:
    assert x.shape == (B, C, H, W) and x.dtype == np.float32
    if "nc" not in _NC_CACHE:
        _NC_CACHE["nc"] = _build_nc()
    nc = _NC_CACHE["nc"]

    bdp, bdn, cb = _host_consts()
    in_maps = []
    for i in range(NCORES):
        shard = x[i * (B // NCORES):(i + 1) * (B // NCORES)]  # (8, 3, 256, 256)
        in_maps.append({
            "x": np.ascontiguousarray(shard).reshape(NCHUNK * 128, FD),
            "bdp": bdp, "bdn": bdn, "cb": cb,
        })
    res = run_bass_kernel_spmd(nc, in_maps, core_ids=list(range(NCORES)))
    parts = []
    for i in range(NCORES):
        f = res.results[i]["out"].astype(np.float64)  # (24, 32) = F_y - F_k
        acc = 8.0 * f
        parts.append(acc.reshape(B // NCORES, C, STEPS))
    return np.concatenate(parts, axis=0).reshape(B, C * STEPS).astype(np.float32)


if __name__ == "__main__":
    rng = np.random.default_rng(0)
    x = rng.random((B, C, H, W), dtype=np.float32)
    y = kernel(x)
    print("kernel out", y.shape, y.dtype, y[:2, :6])


# revision 4
# speedup vs baseline: 1.0824x; 1.0012x over previous
"""Trainium2 Bass kernel for nn_CubECLayr: Euler characteristic curves of
sublevel cubical complexes, batch-data-parallel over 8 NeuronCores.

Algorithm (per core, 24 images of 256x256, 3 chunks of 8 images):
  1. k = ceil(x / DT) per pixel (exact integer bin, magic-number round).
  2. Vertex attribution: signed cell-count delta d per pixel from neighbor
     comparisons in k-space (|d| <= 3); ECC_s = sum_p d_p * [k_p <= s].
  3. Threshold engine via the ramp identity: with y = k + d/8 (exact f16),
         F_v(c) = sum_j min(v_j - c, 0),   c_s = s + 1/2
         ECC_s  = 8 * (F_y(c_s) - F_k(c_s))
     Both families are single-source 4x-mode tensor_scalar passes.
     Reduction is done off-DVE:
       - route A (24 thresholds): PE matmul with +-block-diagonal weights
         sums (w_y - w_k) per image into PSUM [8,512]; ACT drains to [8,1].
       - route C (8 thresholds): ACT Relu+accum computes per-partition
         -F directly (sum relu(c - v)); a final PE matmul reduces to images.
  All intermediate values are integer multiples of 1/8 bounded well below
  2^24/8, so every sum is exact in f16/f32.
"""

import numpy as np

import concourse.bacc as bacc
import concourse.mybir as mybir
from concourse import tile
from concourse.bass_utils import run_bass_kernel_spmd

NCORES = 8
B, C, H, W = 64, 3, 256, 256
IMGS = (B // NCORES) * C          # 24 images per core
CHUNK_IMGS = 8
NCHUNK = IMGS // CHUNK_IMGS       # 3
RB = 16                           # partitions per image
ROWS = H // RB                    # 16 own rows per partition
FD = ROWS * W                     # 4096 own pixels per partition
STEPS = 32
PAD = 1000.0                      # > any real bin; exact in f16
MAGIC = 8388608.0                 # 2^23
HALF = float(np.float32(0.49999997))
F32 = mybir.dt.float32
F16 = mybir.dt.float16
Op = mybir.AluOpType
Act = mybir.ActivationFunctionType

# threshold routing: C-route thresholds handled by ACT alone
C_SET = [3, 7, 11, 15, 19, 23, 27, 31]
A_SET = [s for s in range(STEPS) if s not in C_SET]
NC_ROUTE = len(C_SET)

_NC_CACHE = {}


def _build_nc():
    nc = bacc.Bacc(None, target_bir_lowering=False)
    x_in = nc.dram_tensor("x", [NCHUNK * 128, FD], F32, kind="ExternalInput")
    # bdp: +1 block-diag (f16); bdn: -1 block-diag (f16); bdf: +1 (f32)
    bdp_in = nc.dram_tensor("bdp", [128, CHUNK_IMGS], F32, kind="ExternalInput")
    bdn_in = nc.dram_tensor("bdn", [128, CHUNK_IMGS], F32, kind="ExternalInput")
    # per-threshold ACT bias columns: cb[:, s] = s + 0.5
    cb_in = nc.dram_tensor("cb", [128, STEPS], F32, kind="ExternalInput")
    out = nc.dram_tensor("out", [NCHUNK * CHUNK_IMGS, STEPS], F32,
                         kind="ExternalOutput")

    with tile.TileContext(nc) as tc:
        with (
            tc.tile_pool(name="xp", bufs=2) as xp,
            tc.tile_pool(name="wp", bufs=1) as wp,
            tc.tile_pool(name="hp", bufs=2) as hp,
            tc.tile_pool(name="rp", bufs=4) as rp,
            tc.tile_pool(name="cst", bufs=1) as cst,
            tc.tile_pool(name="pp", bufs=4, space="PSUM") as pp,
            tc.tile_pool(name="pc", bufs=2, space="PSUM") as pc,
        ):
            bdp = cst.tile([128, CHUNK_IMGS], F16)
            bdn = cst.tile([128, CHUNK_IMGS], F16)
            bdf = cst.tile([128, CHUNK_IMGS], F32)
            cb = cst.tile([128, STEPS], F32)
            tmpf = cst.tile([128, CHUNK_IMGS], F32)
            nc.sync.dma_start(out=tmpf[:], in_=bdp_in[:])
            nc.vector.tensor_copy(out=bdp[:], in_=tmpf[:])
            nc.vector.tensor_copy(out=bdf[:], in_=tmpf[:])
            tmpf2 = cst.tile([128, CHUNK_IMGS], F32)
            nc.sync.dma_start(out=tmpf2[:], in_=bdn_in[:])
            nc.vector.tensor_copy(out=bdn[:], in_=tmpf2[:])
            nc.sync.dma_start(out=cb[:], in_=cb_in[:])
            padt = cst.tile([CHUNK_IMGS, W], F16)
            nc.vector.memset(padt[:], PAD)
            adump = cst.tile([128, FD], F16)   # ACT full-output dump
            ddump = cst.tile([8, 512], F32)    # drain dump

            for c in range(NCHUNK):
                xt = xp.tile([128, FD], F32, tag="xt")
                nc.sync.dma_start(out=xt[:], in_=x_in[c * 128:(c + 1) * 128, :])

                # --- bins: k = round(x*31 + (0.5 - eps)) == ceil(x/DT) ---
                nc.vector.tensor_scalar(
                    out=xt[:], in0=xt[:], scalar1=31.0, scalar2=HALF,
                    op0=Op.mult, op1=Op.add)
                # kt rows: 0 = top halo, 1..16 own, 17 = bottom halo (flat cols)
                kt = hp.tile([128, (ROWS + 2) * W], F16, tag="kt")
                nc.vector.tensor_scalar(
                    out=kt[:, W:W + FD], in0=xt[:], scalar1=MAGIC, scalar2=-MAGIC,
                    op0=Op.add, op1=Op.add)
                # halo exchange between partitions (same image), pad at borders
                nc.gpsimd.memset(kt[:, 0:W], PAD)
                nc.gpsimd.memset(kt[:, FD + W:FD + 2 * W], PAD)
                nc.gpsimd.dma_start(out=kt[1:128, 0:W], in_=kt[0:127, FD:FD + W])
                nc.gpsimd.dma_start(out=kt[0:127, FD + W:FD + 2 * W],
                                    in_=kt[1:128, W:2 * W])
                ktop = kt[:, 0:W].rearrange("(a b) w -> a b w", b=RB)
                nc.gpsimd.dma_start(out=ktop[:, 0, :], in_=padt[:])
                kbot = kt[:, FD + W:FD + 2 * W].rearrange("(a b) w -> a b w", b=RB)
                nc.gpsimd.dma_start(out=kbot[:, RB - 1, :], in_=padt[:])

                # --- neighbor comparisons (k-space) ---
                rh = wp.tile([128, FD], F16, tag="rh")
                nc.vector.tensor_tensor(
                    out=rh[:], in0=kt[:, W + 1:W + FD + 1], in1=kt[:, W:W + FD],
                    op=Op.is_ge)
                rv = wp.tile([128, (ROWS + 1) * W], F16, tag="rv")
                nc.vector.tensor_tensor(
                    out=rv[:], in0=kt[:, W:], in1=kt[:, 0:(ROWS + 1) * W],
                    op=Op.is_ge)
                khe = wp.tile([128, (ROWS + 2) * W], F16, tag="khe")
                nc.vector.tensor_tensor(
                    out=khe[:, 0:(ROWS + 2) * W - 1],
                    in0=kt[:, 0:(ROWS + 2) * W - 1], in1=kt[:, 1:(ROWS + 2) * W],
                    op=Op.max)
                nc.gpsimd.memset(
                    khe[:, (ROWS + 2) * W - 1:(ROWS + 2) * W], PAD)
                ut = wp.tile([128, (ROWS + 1) * W], F16, tag="ut")
                nc.vector.tensor_tensor(
                    out=ut[:], in0=khe[:, W:], in1=khe[:, 0:(ROWS + 1) * W],
                    op=Op.is_ge)
                cc = wp.tile([128, FD], F16, tag="cc")
                nc.vector.tensor_tensor(
                    out=cc[:], in0=ut[:, W:], in1=ut[:, 0:FD], op=Op.subtract)
                cc3 = cc[:].rearrange("p (r w) -> p r w", w=W)
                nc.gpsimd.memset(cc3[:, :, W - 1:W], 0.0)
                tt = wp.tile([128, FD], F16, tag="tt")
                nc.vector.tensor_tensor(out=tt[:], in0=rh[:], in1=cc[:], op=Op.mult)

                # --- delta assembly: dl = rv_b - rv_a + t - t_left - cc ---
                dl = wp.tile([128, FD], F16, tag="dl")
                nc.vector.tensor_tensor(
                    out=dl[:], in0=rv[:, W:], in1=rv[:, 0:FD], op=Op.subtract)
                nc.vector.tensor_tensor(out=dl[:], in0=dl[:], in1=tt[:], op=Op.add)
                nc.vector.tensor_tensor(
                    out=dl[:, 1:FD], in0=dl[:, 1:FD], in1=tt[:, 0:FD - 1],
                    op=Op.subtract)
                nc.vector.tensor_tensor(
                    out=dl[:], in0=dl[:], in1=cc[:], op=Op.subtract)

                # --- y = k + d/8 (exact in f16) ---
                yt = hp.tile([128, FD], F16, tag="yt")
                nc.vector.tensor_scalar(
                    out=yt[:], in0=dl[:], scalar1=0.125, scalar2=0.0,
                    op0=Op.mult, op1=Op.add)
                nc.vector.tensor_tensor(
                    out=yt[:], in0=yt[:], in1=kt[:, W:W + FD], op=Op.add)
                kv = kt[:, W:W + FD]

                # --- output tile for this chunk ---
                ft = hp.tile([8, STEPS], F32, tag="ft")
                # C-route accum columns: [y0..y7 | k0..k7]
                colt = hp.tile([128, 2 * NC_ROUTE], F32, tag="colt")

                # --- threshold engine, interleaved in s-order ---
                # route A: DVE ramp -> PE +-bd matmul -> ACT drain
                # route C: ACT Relu+accum (sum relu(c - v) = -F_v)
                for s in range(STEPS):
                    cs = s + 0.5
                    if s in C_SET:
                        i = C_SET.index(s)
                        nc.scalar.activation(
                            out=adump[:], in_=yt[:], func=Act.Relu,
                            bias=cb[:, s:s + 1], scale=-1.0,
                            accum_out=colt[:, i:i + 1])
                        nc.scalar.activation(
                            out=adump[:], in_=kv, func=Act.Relu,
                            bias=cb[:, s:s + 1], scale=-1.0,
                            accum_out=colt[:, NC_ROUTE + i:NC_ROUTE + i + 1])
                        continue
                    wy = rp.tile([128, FD], F16, tag="wy")
                    nc.vector.tensor_scalar(
                        out=wy[:], in0=yt[:], scalar1=-cs, scalar2=0.0,
                        op0=Op.add, op1=Op.min)
                    wk = rp.tile([128, FD], F16, tag="wk")
                    nc.vector.tensor_scalar(
                        out=wk[:], in0=kv, scalar1=-cs, scalar2=0.0,
                        op0=Op.add, op1=Op.min)
                    psum = pp.tile([8, 512], F32, tag="ps")
                    wy3 = wy[:].rearrange("p (h n) -> p h n", n=512)
                    wk3 = wk[:].rearrange("p (h n) -> p h n", n=512)
                    for h in range(8):
                        nc.tensor.matmul(psum[:], bdp[:], wy3[:, h, :],
                                         start=(h == 0), stop=False)
                    for h in range(8):
                        nc.tensor.matmul(psum[:], bdn[:], wk3[:, h, :],
                                         start=False, stop=(h == 7))
                    nc.scalar.activation(
                        out=ddump[:], in_=psum[:], func=Act.Copy,
                        accum_out=ft[:, s:s + 1])
                # D = colK - colY = F_y - F_k  (per partition)
                dcol = hp.tile([128, NC_ROUTE], F32, tag="dcol")
                nc.vector.tensor_tensor(
                    out=dcol[:], in0=colt[:, NC_ROUTE:2 * NC_ROUTE],
                    in1=colt[:, 0:NC_ROUTE], op=Op.subtract)
                psc = pc.tile([8, NC_ROUTE], F32, tag="psc")
                nc.tensor.matmul(psc[:], bdf[:], dcol[:], start=True, stop=True)
                ft4 = ft[:].rearrange("p (a b) -> p a b", b=4)
                nc.scalar.activation(
                    out=ft4[:, :, 3], in_=psc[:], func=Act.Copy)

                nc.sync.dma_start(
                    out=out[c * CHUNK_IMGS:(c + 1) * CHUNK_IMGS, :], in_=ft[:])

    nc.finalize()
    return nc


def _host_consts():
    bdp = np.zeros((128, CHUNK_IMGS), dtype=np.float32)
    for p in range(128):
        bdp[p, p // RB] = 1.0
    bdn = -bdp
    cb = np.tile((np.arange(STEPS, dtype=np.float32) + 0.5)[None, :], (128, 1))
    return bdp, bdn, cb


def kernel(x: np.ndarray) -> np.ndarray:
    assert x.shape == (B, C, H, W) and x.dtype == np.float32
    if "nc" not in _NC_CACHE:
        _NC_CACHE["nc"] = _build_nc()
    nc = _NC_CACHE["nc"]

    bdp, bdn, cb = _host_consts()
    in_maps = []
    for i in range(NCORES):
        shard = x[i * (B // NCORES):(i + 1) * (B // NCORES)]  # (8, 3, 256, 256)
        in_maps.append({
            "x": np.ascontiguousarray(shard).reshape(NCHUNK * 128, FD),
            "bdp": bdp, "bdn": bdn, "cb": cb,
        })
    res = run_bass_kernel_spmd(nc, in_maps, core_ids=list(range(NCORES)))
    parts = []
    for i in range(NCORES):
        f = res.results[i]["out"].astype(np.float64)  # (24, 32) = F_y - F_k
        acc = 8.0 * f
        parts.append(acc.reshape(B // NCORES, C, STEPS))
    return np.concatenate(parts, axis=0).reshape(B, C * STEPS).astype(np.float32)


if __name__ == "__main__":
    rng = np.random.default_rng(0)
    x = rng.random((B, C, H, W), dtype=np.float32)
    y = kernel(x)
    print("kernel out", y.shape, y.dtype, y[:2, :6])


# revision 11
# speedup vs baseline: 1.1422x; 1.0553x over previous
"""Trainium2 Bass kernel for nn_CubECLayr: Euler characteristic curves of
sublevel cubical complexes, batch-data-parallel over 8 NeuronCores.

Algorithm (per core, 24 images of 256x256, 3 chunks of 8 images):
  1. k = ceil(x / DT) per pixel (exact integer bin, magic-number round).
  2. Vertex attribution: signed cell-count delta d per pixel from neighbor
     comparisons in k-space (|d| <= 3); ECC_s = sum_p d_p * [k_p <= s].
  3. Threshold engine via the ramp identity: with y = k + d/8 (exact f16),
         F_v(c) = sum_j min(v_j - c, 0),   c_s = s + 1/2
         ECC_s  = 8 * (F_y(c_s) - F_k(c_s))
     Both families are single-source 4x-mode tensor_scalar passes.
     Reduction is done off-DVE:
       - route A: PE matmul with +-block-diagonal weights sums (w_y - w_k)
         per image into an 8-partition band of a [128,512] PSUM tile; one
         ACT Copy+accum pass drains 16 thresholds at once.
       - route C: ACT Relu+accum computes per-partition -F directly
         (sum relu(c - v)); a final PE matmul reduces to images.
  All intermediate values are integer multiples of 1/8 bounded well below
  2^24/8, so every sum is exact in f16/f32.
"""

import numpy as np

import concourse.bacc as bacc
import concourse.mybir as mybir
from concourse import tile
from concourse.bass_utils import run_bass_kernel_spmd

NCORES = 8
B, C, H, W = 64, 3, 256, 256
IMGS = (B // NCORES) * C          # 24 images per core
CHUNK_IMGS = 8
NCHUNK = IMGS // CHUNK_IMGS       # 3
RB = 16                           # partitions per image
ROWS = H // RB                    # 16 own rows per partition
FD = ROWS * W                     # 4096 own pixels per partition
STEPS = 32
PAD = 1000.0                      # > any real bin; exact in f16
MAGIC = 8388608.0                 # 2^23
HALF = float(np.float32(0.49999997))
F32 = mybir.dt.float32
F16 = mybir.dt.float16
Op = mybir.AluOpType
Act = mybir.ActivationFunctionType

# threshold routing: C-route thresholds handled by ACT alone
C_SET = [2, 5, 8, 11, 14, 17, 20, 23, 26, 29, 31]
A_SET = [s for s in range(STEPS) if s not in C_SET]
NC_ROUTE = len(C_SET)             # 11
NA = len(A_SET)                   # 21
NDRAIN = NA // 3                  # 3 thresholds per psum tile (bases 0/32/64)

_NC_CACHE = {}


def _build_nc():
    nc = bacc.Bacc(None, target_bir_lowering=False)
    x_in = nc.dram_tensor("x", [NCHUNK * 128, FD], F32, kind="ExternalInput")
    bdp_in = nc.dram_tensor("bdp", [128, 32], F32, kind="ExternalInput")
    bdn_in = nc.dram_tensor("bdn", [128, 32], F32, kind="ExternalInput")
    cb_in = nc.dram_tensor("cb", [128, STEPS], F32, kind="ExternalInput")
    outa = nc.dram_tensor("outa", [NCHUNK * 96, NDRAIN], F32,
                          kind="ExternalOutput")
    outc = nc.dram_tensor("outc", [NCHUNK * CHUNK_IMGS, NC_ROUTE], F32,
                          kind="ExternalOutput")

    with tile.TileContext(nc) as tc:
        with (
            tc.tile_pool(name="xp", bufs=2) as xp,
            tc.tile_pool(name="wp", bufs=1) as wp,
            tc.tile_pool(name="hp", bufs=2) as hp,
            tc.tile_pool(name="rp", bufs=2) as rp,
            tc.tile_pool(name="cst", bufs=1) as cst,
            tc.tile_pool(name="pp", bufs=4, space="PSUM") as pp,
            tc.tile_pool(name="pc", bufs=2, space="PSUM") as pc,
        ):
            bdp = cst.tile([128, 32], F16)
            bdn = cst.tile([128, 32], F16)
            bdf = cst.tile([128, CHUNK_IMGS], F32)
            cb = cst.tile([128, STEPS], F32)
            tmpf = cst.tile([128, 32], F32)
            nc.sync.dma_start(out=tmpf[:], in_=bdp_in[:])
            nc.vector.tensor_copy(out=bdp[:], in_=tmpf[:])
            nc.vector.tensor_copy(out=bdf[:], in_=tmpf[:, 0:CHUNK_IMGS])
            tmpf2 = cst.tile([128, 32], F32)
            nc.sync.dma_start(out=tmpf2[:], in_=bdn_in[:])
            nc.vector.tensor_copy(out=bdn[:], in_=tmpf2[:])
            nc.sync.dma_start(out=cb[:], in_=cb_in[:])
            adump = cst.tile([128, FD], F16)   # ACT full-output dump
            ddump = cst.tile([96, 512], F32)   # drain dump

            for c in range(NCHUNK):
                xt = xp.tile([128, FD], F32, tag="xt")
                nc.sync.dma_start(out=xt[:], in_=x_in[c * 128:(c + 1) * 128, :])

                # --- bins: k = round(x*31 + (0.5 - eps)) == ceil(x/DT) ---
                nc.vector.tensor_scalar(
                    out=xt[:], in0=xt[:], scalar1=31.0, scalar2=HALF,
                    op0=Op.mult, op1=Op.add)
                kt = hp.tile([128, (ROWS + 2) * W], F16, tag="kt")
                nc.vector.tensor_scalar(
                    out=kt[:, W:W + FD], in0=xt[:], scalar1=MAGIC, scalar2=-MAGIC,
                    op0=Op.add, op1=Op.add)
                # halo exchange between partitions (same image), pad at borders
                nc.gpsimd.memset(kt[:, 0:W], PAD)
                nc.gpsimd.memset(kt[:, FD + W:FD + 2 * W], PAD)
                for i in range(CHUNK_IMGS):
                    p0 = RB * i
                    nc.sync.dma_start(
                        out=kt[p0 + 1:p0 + RB, 0:W],
                        in_=kt[p0:p0 + RB - 1, FD:FD + W])
                    nc.sync.dma_start(
                        out=kt[p0:p0 + RB - 1, FD + W:FD + 2 * W],
                        in_=kt[p0 + 1:p0 + RB, W:2 * W])

                # --- neighbor comparisons (k-space) ---
                rh = wp.tile([128, FD], F16, tag="rh")
                nc.vector.tensor_tensor(
                    out=rh[:], in0=kt[:, W + 1:W + FD + 1], in1=kt[:, W:W + FD],
                    op=Op.is_ge)
                rv = wp.tile([128, (ROWS + 1) * W], F16, tag="rv")
                nc.vector.tensor_tensor(
                    out=rv[:], in0=kt[:, W:], in1=kt[:, 0:(ROWS + 1) * W],
                    op=Op.is_ge)
                khe = wp.tile([128, (ROWS + 2) * W], F16, tag="khe")
                nc.vector.tensor_tensor(
                    out=khe[:, 0:(ROWS + 2) * W - 1],
                    in0=kt[:, 0:(ROWS + 2) * W - 1], in1=kt[:, 1:(ROWS + 2) * W],
                    op=Op.max)
                nc.gpsimd.memset(
                    khe[:, (ROWS + 2) * W - 1:(ROWS + 2) * W], PAD)
                ut = wp.tile([128, (ROWS + 1) * W], F16, tag="ut")
                nc.vector.tensor_tensor(
                    out=ut[:], in0=khe[:, W:], in1=khe[:, 0:(ROWS + 1) * W],
                    op=Op.is_ge)
                cc = wp.tile([128, FD], F16, tag="cc")
                nc.vector.tensor_tensor(
                    out=cc[:], in0=ut[:, W:], in1=ut[:, 0:FD], op=Op.subtract)
                cc3 = cc[:].rearrange("p (r w) -> p r w", w=W)
                nc.gpsimd.memset(cc3[:, :, W - 1:W], 0.0)
                tt = rh  # in-place: tt = rh * cc
                nc.vector.tensor_tensor(out=tt[:], in0=rh[:], in1=cc[:], op=Op.mult)

                # --- delta assembly: dl = rv_b - rv_a + t - t_left - cc ---
                dl = wp.tile([128, FD], F16, tag="dl")
                nc.vector.tensor_tensor(
                    out=dl[:], in0=rv[:, W:], in1=rv[:, 0:FD], op=Op.subtract)
                nc.vector.tensor_tensor(out=dl[:], in0=dl[:], in1=tt[:], op=Op.add)
                nc.vector.tensor_tensor(
                    out=dl[:, 1:FD], in0=dl[:, 1:FD], in1=tt[:, 0:FD - 1],
                    op=Op.subtract)
                nc.vector.tensor_tensor(
                    out=dl[:], in0=dl[:], in1=cc[:], op=Op.subtract)

                # --- y = k + d/8 (exact in f16) ---
                yt = hp.tile([128, FD], F16, tag="yt")
                nc.vector.tensor_scalar(
                    out=yt[:], in0=dl[:], scalar1=0.125, scalar2=0.0,
                    op0=Op.mult, op1=Op.add)
                nc.vector.tensor_tensor(
                    out=yt[:], in0=yt[:], in1=kt[:, W:W + FD], op=Op.add)
                kv = kt[:, W:W + FD]

                ft = hp.tile([8, NC_ROUTE], F32, tag="ft")
                colt = hp.tile([128, 2 * NC_ROUTE], F32, tag="colt")
                cola = hp.tile([96, NDRAIN], F32, tag="cola")

                # --- threshold engine, interleaved in s-order ---
                # A-route: 3 thresholds share one [72,512] psum tile at
                # partition bases 0/32/64; one ACT pass drains all three.
                ps = None
                for s in range(STEPS):
                    cs = s + 0.5
                    if s in C_SET:
                        i = C_SET.index(s)
                        nc.scalar.activation(
                            out=adump[:], in_=yt[:], func=Act.Relu,
                            bias=cb[:, s:s + 1], scale=-1.0,
                            accum_out=colt[:, i:i + 1])
                        nc.scalar.activation(
                            out=adump[:], in_=kv, func=Act.Relu,
                            bias=cb[:, s:s + 1], scale=-1.0,
                            accum_out=colt[:, NC_ROUTE + i:NC_ROUTE + i + 1])
                        continue
                    j = A_SET.index(s)
                    d, m = j // 3, j % 3
                    if m == 0:
                        ps = pp.tile([96, 512], F32, tag="ps")
                    reg = ps[32 * m:32 * m + 32, :]
                    wy = rp.tile([128, FD], F16, tag="wy")
                    nc.vector.tensor_scalar(
                        out=wy[:], in0=yt[:], scalar1=-cs, scalar2=0.0,
                        op0=Op.add, op1=Op.min)
                    wk = rp.tile([128, FD], F16, tag="wk")
                    nc.vector.tensor_scalar(
                        out=wk[:], in0=kv, scalar1=-cs, scalar2=0.0,
                        op0=Op.add, op1=Op.min)
                    wy3 = wy[:].rearrange("p (h n) -> p h n", n=512)
                    wk3 = wk[:].rearrange("p (h n) -> p h n", n=512)
                    for h in range(8):
                        nc.tensor.matmul(reg, bdp[:], wy3[:, h, :],
                                         start=(h == 0), stop=False)
                    for h in range(8):
                        nc.tensor.matmul(reg, bdn[:], wk3[:, h, :],
                                         start=False, stop=(h == 7))
                    if m == 2:
                        nc.scalar.activation(
                            out=ddump[:], in_=ps[:], func=Act.Copy,
                            accum_out=cola[:, d:d + 1])

                # C-route: D = colK - colY = F_y - F_k per partition -> images
                dcol = hp.tile([128, NC_ROUTE], F32, tag="dcol")
                nc.vector.tensor_tensor(
                    out=dcol[:], in0=colt[:, NC_ROUTE:2 * NC_ROUTE],
                    in1=colt[:, 0:NC_ROUTE], op=Op.subtract)
                psc = pc.tile([8, NC_ROUTE], F32, tag="psc")
                nc.tensor.matmul(psc[:], bdf[:], dcol[:], start=True, stop=True)
                nc.scalar.activation(out=ft[:], in_=psc[:], func=Act.Copy)

                nc.sync.dma_start(
                    out=outa[c * 96:(c + 1) * 96, :], in_=cola[:])
                nc.sync.dma_start(
                    out=outc[c * CHUNK_IMGS:(c + 1) * CHUNK_IMGS, :], in_=ft[:])

    nc.finalize()
    return nc


def _host_consts():
    bd1 = np.zeros((128, CHUNK_IMGS), dtype=np.float32)
    for p in range(128):
        bd1[p, p // RB] = 1.0
    bdp = np.tile(bd1, (1, 4))
    bdn = -bdp
    cb = np.tile((np.arange(STEPS, dtype=np.float32) + 0.5)[None, :], (128, 1))
    return bdp, bdn, cb


def _assemble(fa, fc):
    """fa: (NCHUNK*128, NDRAIN); fc: (NCHUNK*8, NC_ROUTE) -> acc (24, 32)."""
    acc = np.zeros((IMGS, STEPS), dtype=np.float64)
    fa = fa.reshape(NCHUNK, 96, NDRAIN).astype(np.float64)
    fc = fc.reshape(NCHUNK, CHUNK_IMGS, NC_ROUTE).astype(np.float64)
    for c in range(NCHUNK):
        for j, s in enumerate(A_SET):
            d, m = j // 3, j % 3
            for i in range(CHUNK_IMGS):
                acc[c * CHUNK_IMGS + i, s] = 8.0 * fa[c, 32 * m + i, d]
        for j, s in enumerate(C_SET):
            acc[c * CHUNK_IMGS:(c + 1) * CHUNK_IMGS, s] = 8.0 * fc[c, :, j]
    return acc


def kernel(x: np.ndarray) -> np.ndarray:
    assert x.shape == (B, C, H, W) and x.dtype == np.float32
    if "nc" not in _NC_CACHE:
        _NC_CACHE["nc"] = _build_nc()
    nc = _NC_CACHE["nc"]

    bdp, bdn, cb = _host_consts()
    in_maps = []
    for i in range(NCORES):
        shard = x[i * (B // NCORES):(i + 1) * (B // NCORES)]  # (8, 3, 256, 256)
        in_maps.append({
            "x": np.ascontiguousarray(shard).reshape(NCHUNK * 128, FD),
            "bdp": bdp, "bdn": bdn, "cb": cb,
        })
    res = run_bass_kernel_spmd(nc, in_maps, core_ids=list(range(NCORES)))
    parts = []
    for i in range(NCORES):
        acc = _assemble(res.results[i]["outa"], res.results[i]["outc"])
        parts.append(acc.reshape(B // NCORES, C, STEPS))
    return np.concatenate(parts, axis=0).reshape(B, C * STEPS).astype(np.float32)


if __name__ == "__main__":
    rng = np.random.default_rng(0)
    x = rng.random((B, C, H, W), dtype=np.float32)
    y = kernel(x)
    print("kernel out", y.shape, y.dtype, y[:2, :6])


# revision 12
# speedup vs baseline: 1.2688x; 1.1108x over previous
"""Trainium2 Bass kernel for nn_CubECLayr: Euler characteristic curves of
sublevel cubical complexes, batch-data-parallel over 8 NeuronCores.

Algorithm (per core, 24 images of 256x256, 3 chunks of 8 images):
  1. k = ceil(x / DT) per pixel (exact integer bin, magic-number round).
  2. Vertex attribution: signed cell-count delta d per pixel from neighbor
     comparisons in k-space (|d| <= 3); ECC_s = sum_p d_p * [k_p <= s].
  3. Threshold engine via the ramp identity: with y = k + d/8 (exact f16),
         F_v(c) = sum_j min(v_j - c, 0),   c_s = s + 1/2
         ECC_s  = 8 * (F_y(c_s) - F_k(c_s))
     Both families are single-source 4x-mode tensor_scalar passes.
     Reduction is done off-DVE:
       - route A: PE matmul with +-block-diagonal weights sums (w_y - w_k)
         per image into an 8-partition band of a [128,512] PSUM tile; one
         ACT Copy+accum pass drains 16 thresholds at once.
       - route C: ACT Relu+accum computes per-partition -F directly
         (sum relu(c - v)); a final PE matmul reduces to images.
  All intermediate values are integer multiples of 1/8 bounded well below
  2^24/8, so every sum is exact in f16/f32.
"""

import numpy as np

import concourse.bacc as bacc
import concourse.mybir as mybir
from concourse import tile
from concourse.bass_utils import run_bass_kernel_spmd

NCORES = 8
B, C, H, W = 64, 3, 256, 256
IMGS = (B // NCORES) * C          # 24 images per core
CHUNK_IMGS = 8
NCHUNK = IMGS // CHUNK_IMGS       # 3
RB = 16                           # partitions per image
ROWS = H // RB                    # 16 own rows per partition
FD = ROWS * W                     # 4096 own pixels per partition
STEPS = 32
PAD = 1000.0                      # > any real bin; exact in f16
MAGIC = 8388608.0                 # 2^23
HALF = float(np.float32(0.49999997))
F32 = mybir.dt.float32
F16 = mybir.dt.float16
Op = mybir.AluOpType
Act = mybir.ActivationFunctionType

# threshold routing: C-route thresholds handled by ACT alone
C_SET = [3, 7, 11, 15, 19, 23, 27, 29, 31]
A_SET = [s for s in range(STEPS) if s not in C_SET]
NC_ROUTE = len(C_SET)             # 9
NA = len(A_SET)                   # 23
NDRAIN = (NA + 2) // 3            # 3 thresholds per psum tile (bases 0/32/64)
LAST_BANDS = NA - 3 * (NDRAIN - 1)

_NC_CACHE = {}


def _build_nc():
    nc = bacc.Bacc(None, target_bir_lowering=False)
    x_in = nc.dram_tensor("x", [NCHUNK * 128, FD], F32, kind="ExternalInput")
    bdp_in = nc.dram_tensor("bdp", [128, 32], F32, kind="ExternalInput")
    bdn_in = nc.dram_tensor("bdn", [128, 32], F32, kind="ExternalInput")
    cb_in = nc.dram_tensor("cb", [128, STEPS], F32, kind="ExternalInput")
    outa = nc.dram_tensor("outa", [NCHUNK * 96, NDRAIN], F32,
                          kind="ExternalOutput")
    outc = nc.dram_tensor("outc", [NCHUNK * CHUNK_IMGS, NC_ROUTE], F32,
                          kind="ExternalOutput")

    with tile.TileContext(nc) as tc:
        with (
            tc.tile_pool(name="xp", bufs=2) as xp,
            tc.tile_pool(name="wp", bufs=1) as wp,
            tc.tile_pool(name="hp", bufs=2) as hp,
            tc.tile_pool(name="rp", bufs=3) as rp,
            tc.tile_pool(name="cst", bufs=1) as cst,
            tc.tile_pool(name="pp", bufs=5, space="PSUM") as pp,
            tc.tile_pool(name="pc", bufs=2, space="PSUM") as pc,
        ):
            bdp = cst.tile([128, 32], F16)
            bdn = cst.tile([128, 32], F16)
            bdf = cst.tile([128, CHUNK_IMGS], F32)
            cb = cst.tile([128, STEPS], F32)
            tmpf = cst.tile([128, 32], F32)
            nc.sync.dma_start(out=tmpf[:], in_=bdp_in[:])
            nc.vector.tensor_copy(out=bdp[:], in_=tmpf[:])
            nc.vector.tensor_copy(out=bdf[:], in_=tmpf[:, 0:CHUNK_IMGS])
            tmpf2 = cst.tile([128, 32], F32)
            nc.sync.dma_start(out=tmpf2[:], in_=bdn_in[:])
            nc.vector.tensor_copy(out=bdn[:], in_=tmpf2[:])
            nc.sync.dma_start(out=cb[:], in_=cb_in[:])
            adump = cst.tile([128, FD], F16)   # ACT full-output dump
            ddump = cst.tile([96, 512], F32)   # drain dump

            for c in range(NCHUNK):
                xt = xp.tile([128, FD], F32, tag="xt")
                nc.sync.dma_start(out=xt[:], in_=x_in[c * 128:(c + 1) * 128, :])

                # --- bins: k = round(x*31 + (0.5 - eps)) == ceil(x/DT) ---
                nc.vector.tensor_scalar(
                    out=xt[:], in0=xt[:], scalar1=31.0, scalar2=HALF,
                    op0=Op.mult, op1=Op.add)
                kt = hp.tile([128, (ROWS + 2) * W], F16, tag="kt")
                nc.vector.tensor_scalar(
                    out=kt[:, W:W + FD], in0=xt[:], scalar1=MAGIC, scalar2=-MAGIC,
                    op0=Op.add, op1=Op.add)
                # halo exchange between partitions (same image), pad at borders
                nc.gpsimd.memset(kt[:, 0:W], PAD)
                nc.gpsimd.memset(kt[:, FD + W:FD + 2 * W], PAD)
                for i in range(CHUNK_IMGS):
                    p0 = RB * i
                    nc.sync.dma_start(
                        out=kt[p0 + 1:p0 + RB, 0:W],
                        in_=kt[p0:p0 + RB - 1, FD:FD + W])
                    nc.sync.dma_start(
                        out=kt[p0:p0 + RB - 1, FD + W:FD + 2 * W],
                        in_=kt[p0 + 1:p0 + RB, W:2 * W])

                # --- neighbor comparisons (k-space) ---
                rh = wp.tile([128, FD], F16, tag="rh")
                nc.vector.tensor_tensor(
                    out=rh[:], in0=kt[:, W + 1:W + FD + 1], in1=kt[:, W:W + FD],
                    op=Op.is_ge)
                rv = wp.tile([128, (ROWS + 1) * W], F16, tag="rv")
                nc.vector.tensor_tensor(
                    out=rv[:], in0=kt[:, W:], in1=kt[:, 0:(ROWS + 1) * W],
                    op=Op.is_ge)
                khe = wp.tile([128, (ROWS + 2) * W], F16, tag="khe")
                nc.vector.tensor_tensor(
                    out=khe[:, 0:(ROWS + 2) * W - 1],
                    in0=kt[:, 0:(ROWS + 2) * W - 1], in1=kt[:, 1:(ROWS + 2) * W],
                    op=Op.max)
                nc.gpsimd.memset(
                    khe[:, (ROWS + 2) * W - 1:(ROWS + 2) * W], PAD)
                ut = wp.tile([128, (ROWS + 1) * W], F16, tag="ut")
                nc.vector.tensor_tensor(
                    out=ut[:], in0=khe[:, W:], in1=khe[:, 0:(ROWS + 1) * W],
                    op=Op.is_ge)
                cc = wp.tile([128, FD], F16, tag="cc")
                nc.vector.tensor_tensor(
                    out=cc[:], in0=ut[:, W:], in1=ut[:, 0:FD], op=Op.subtract)
                cc3 = cc[:].rearrange("p (r w) -> p r w", w=W)
                nc.gpsimd.memset(cc3[:, :, W - 1:W], 0.0)
                tt = rh  # in-place: tt = rh * cc
                nc.vector.tensor_tensor(out=tt[:], in0=rh[:], in1=cc[:], op=Op.mult)

                # --- delta assembly: dl = rv_b - rv_a + t - t_left - cc ---
                dl = wp.tile([128, FD], F16, tag="dl")
                nc.vector.tensor_tensor(
                    out=dl[:], in0=rv[:, W:], in1=rv[:, 0:FD], op=Op.subtract)
                nc.vector.tensor_tensor(out=dl[:], in0=dl[:], in1=tt[:], op=Op.add)
                nc.vector.tensor_tensor(
                    out=dl[:, 1:FD], in0=dl[:, 1:FD], in1=tt[:, 0:FD - 1],
                    op=Op.subtract)
                nc.vector.tensor_tensor(
                    out=dl[:], in0=dl[:], in1=cc[:], op=Op.subtract)

                # --- y = k + d/8 (exact in f16) ---
                yt = hp.tile([128, FD], F16, tag="yt")
                nc.vector.tensor_scalar(
                    out=yt[:], in0=dl[:], scalar1=0.125, scalar2=0.0,
                    op0=Op.mult, op1=Op.add)
                nc.vector.tensor_tensor(
                    out=yt[:], in0=yt[:], in1=kt[:, W:W + FD], op=Op.add)
                kv = kt[:, W:W + FD]

                ft = hp.tile([8, NC_ROUTE], F32, tag="ft")
                colt = hp.tile([128, 2 * NC_ROUTE], F32, tag="colt")
                cola = hp.tile([96, NDRAIN], F32, tag="cola")

                # --- threshold engine, interleaved in s-order ---
                # A-route: 3 thresholds share one [72,512] psum tile at
                # partition bases 0/32/64; one ACT pass drains all three.
                ps = None
                for s in range(STEPS):
                    cs = s + 0.5
                    if s in C_SET:
                        i = C_SET.index(s)
                        nc.scalar.activation(
                            out=adump[:], in_=yt[:], func=Act.Relu,
                            bias=cb[:, s:s + 1], scale=-1.0,
                            accum_out=colt[:, i:i + 1])
                        nc.scalar.activation(
                            out=adump[:], in_=kv, func=Act.Relu,
                            bias=cb[:, s:s + 1], scale=-1.0,
                            accum_out=colt[:, NC_ROUTE + i:NC_ROUTE + i + 1])
                        continue
                    j = A_SET.index(s)
                    d, m = j // 3, j % 3
                    nb = 3 if d < NDRAIN - 1 else LAST_BANDS
                    if m == 0:
                        ps = pp.tile([32 * nb, 512], F32, tag="ps")
                    reg = ps[32 * m:32 * m + 32, :]
                    wy = rp.tile([128, FD], F16, tag="wy")
                    nc.vector.tensor_scalar(
                        out=wy[:], in0=yt[:], scalar1=-cs, scalar2=0.0,
                        op0=Op.add, op1=Op.min)
                    wk = rp.tile([128, FD], F16, tag="wk")
                    nc.vector.tensor_scalar(
                        out=wk[:], in0=kv, scalar1=-cs, scalar2=0.0,
                        op0=Op.add, op1=Op.min)
                    wy3 = wy[:].rearrange("p (h n) -> p h n", n=512)
                    wk3 = wk[:].rearrange("p (h n) -> p h n", n=512)
                    for h in range(8):
                        nc.tensor.matmul(reg, bdp[:], wy3[:, h, :],
                                         start=(h == 0), stop=False)
                    for h in range(8):
                        nc.tensor.matmul(reg, bdn[:], wk3[:, h, :],
                                         start=False, stop=(h == 7))
                    if m == nb - 1:
                        nc.scalar.activation(
                            out=ddump[0:32 * nb, :], in_=ps[:], func=Act.Copy,
                            accum_out=cola[0:32 * nb, d:d + 1])

                # C-route: D = colK - colY = F_y - F_k per partition -> images
                dcol = hp.tile([128, NC_ROUTE], F32, tag="dcol")
                nc.vector.tensor_tensor(
                    out=dcol[:], in0=colt[:, NC_ROUTE:2 * NC_ROUTE],
                    in1=colt[:, 0:NC_ROUTE], op=Op.subtract)
                psc = pc.tile([8, NC_ROUTE], F32, tag="psc")
                nc.tensor.matmul(psc[:], bdf[:], dcol[:], start=True, stop=True)
                nc.scalar.activation(out=ft[:], in_=psc[:], func=Act.Copy)

                nc.sync.dma_start(
                    out=outa[c * 96:(c + 1) * 96, :], in_=cola[:])
                nc.sync.dma_start(
                    out=outc[c * CHUNK_IMGS:(c + 1) * CHUNK_IMGS, :], in_=ft[:])

    nc.finalize()
    return nc


def _host_consts():
    bd1 = np.zeros((128, CHUNK_IMGS), dtype=np.float32)
    for p in range(128):
        bd1[p, p // RB] = 1.0
    bdp = np.tile(bd1, (1, 4))
    bdn = -bdp
    cb = np.tile((np.arange(STEPS, dtype=np.float32) + 0.5)[None, :], (128, 1))
    return bdp, bdn, cb


def _assemble(fa, fc):
    """fa: (NCHUNK*128, NDRAIN); fc: (NCHUNK*8, NC_ROUTE) -> acc (24, 32)."""
    acc = np.zeros((IMGS, STEPS), dtype=np.float64)
    fa = fa.reshape(NCHUNK, 96, NDRAIN).astype(np.float64)
    fc = fc.reshape(NCHUNK, CHUNK_IMGS, NC_ROUTE).astype(np.float64)
    for c in range(NCHUNK):
        for j, s in enumerate(A_SET):
            d, m = j // 3, j % 3
            for i in range(CHUNK_IMGS):
                acc[c * CHUNK_IMGS + i, s] = 8.0 * fa[c, 32 * m + i, d]
        for j, s in enumerate(C_SET):
            acc[c * CHUNK_IMGS:(c + 1) * CHUNK_IMGS, s] = 8.0 * fc[c, :, j]
    return acc


def kernel(x: np.ndarray) -> np.ndarray:
    assert x.shape == (B, C, H, W) and x.dtype == np.float32
    if "nc" not in _NC_CACHE:
        _NC_CACHE["nc"] = _build_nc()
    nc = _NC_CACHE["nc"]

    bdp, bdn, cb = _host_consts()
    in_maps = []
    for i in range(NCORES):
        shard = x[i * (B // NCORES):(i + 1) * (B // NCORES)]  # (8, 3, 256, 256)
        in_maps.append({
            "x": np.ascontiguousarray(shard).reshape(NCHUNK * 128, FD),
            "bdp": bdp, "bdn": bdn, "cb": cb,
        })
    res = run_bass_kernel_spmd(nc, in_maps, core_ids=list(range(NCORES)))
    parts = []
    for i in range(NCORES):
        acc = _assemble(res.results[i]["outa"], res.results[i]["outc"])
        parts.append(acc.reshape(B // NCORES, C, STEPS))
    return np.concatenate(parts, axis=0).reshape(B, C * STEPS).astype(np.float32)


if __name__ == "__main__":
    rng = np.random.default_rng(0)
    x = rng.random((B, C, H, W), dtype=np.float32)
    y = kernel(x)
    print("kernel out", y.shape, y.dtype, y[:2, :6])
